# revision 27
# baseline (speedup 1.0000x reference)
"""MultiHeadDiffAttention Trainium2 kernel (8 NeuronCores), v3.

Sharding: data-parallel over batch (B=2 -> 2 groups of 4 cores), tensor-parallel
over heads within a group (16 heads -> 4 heads/core).

Pipeline (per core, SPMD uniform):
  - all storage bf16 (x, weights, K/Q/V tiles, exp weights, attention out);
    matmul accumulation f32 in PSUM.
  - qt-outer / head-inner attention: per 512-query tile, per head, stream 16
    key chunks: scores matmul -> exp (the ONLY steady ACT work, [128,1024]
    per chunk) -> P^T@V matmul with a ones column riding along for the
    softmax sums.
  - combine: evacuate raw PV sums fast (releases the PSUM accumulator),
    reciprocal the sums row, lambda-scale branch 2, gpsimd broadcast, two
    multiplies + subtract on DVE, ACT copy places the head rows into otc.
  - per-qt Wo partial [512, D] bf16 -> ReduceScatter over the 4-core group
    (issued early in the next qt, overlapped) -> per-qt LayerNorm -> out.
  - PSUM: pool A [128,1024]x2 (scores), pool B [128,1024]x1 (PV accum),
    pool C [128,512]x2 (projections/Wo) = 8 banks.
  - projections/Wo/RS/LN ride a deadline-sorted step queue, popped one
    ~0.85us step per attention chunk so the exp stream never starves and
    the in-order engine queues avoid priority inversion.

Output rows per core: row (qt*128 + i) = batch row (qt*512 + rank*128 + i).
"""

import math
from contextlib import ExitStack

import numpy as np
import ml_dtypes

import concourse.bass as bass
import concourse.mybir as mybir
import concourse.tile as tile
from concourse import bacc
from concourse import bass_utils

F32 = mybir.dt.float32
BF16 = mybir.dt.bfloat16

B = 2
S = 2048
D = 1024
NH = 16
HD = 64
N_CORES = 8
GRP = N_CORES // B  # 4 cores per batch group
NH_LOC = NH // GRP  # 4 heads per core
DQ = NH_LOC * 2 * HD  # 512 local q/k projection width
DV = NH_LOC * HD  # 256 local v projection width
LAYER_IDX = 12
LAMBDA_INIT = 0.8 - 0.6 * math.exp(-0.3 * (LAYER_IDX - 1))
LN_EPS = 1e-5
SCALE = HD ** (-0.5)

KC = D // 128  # 8 contraction chunks for projections
SB = S // 128  # 16 key chunks
NQT = S // 512  # 4 query tiles

_CACHE = {}
FAST_DEFAULT = False


def _build(repeat=1, single=False, fast=False, nocc=False):
    nc = bacc.Bacc("TRN2", target_bir_lowering=False, debug=False,
                   num_devices=1 if single else N_CORES)

    xT = nc.dram_tensor("xT", [D, S], BF16, kind="ExternalInput").ap()
    wq = nc.dram_tensor("wq", [D, DQ], BF16, kind="ExternalInput").ap()
    wk = nc.dram_tensor("wk", [D, DQ], BF16, kind="ExternalInput").ap()
    wv = nc.dram_tensor("wv", [D, DV], BF16, kind="ExternalInput").ap()
    wo = nc.dram_tensor("wo", [DV, D], BF16, kind="ExternalInput").ap()
    lam = nc.dram_tensor("lam", [1, 1], F32, kind="ExternalInput").ap()
    gamma = nc.dram_tensor("gamma", [1, D], F32, kind="ExternalInput").ap()
    beta = nc.dram_tensor("beta", [1, D], F32, kind="ExternalInput").ap()
    out = nc.dram_tensor("out", [S // 4, D], F32, kind="ExternalOutput").ap()

    with tile.TileContext(nc) as tc, ExitStack() as ctx:
        sb = ctx.enter_context(tc.tile_pool(name="sb", bufs=1))
        ps = ctx.enter_context(tc.tile_pool(name="ps", bufs=1, space="PSUM"))
        dram = ctx.enter_context(tc.tile_pool(name="dram", bufs=1, space="DRAM"))

        # ---- constants ----
        lam_sb = sb.tile([128, 1], F32, tag="lam")
        nc.sync.dma_start(out=lam_sb, in_=lam.to_broadcast([128, 1]))
        gamma_sb = sb.tile([128, D], F32, tag="gamma")
        nc.sync.dma_start(out=gamma_sb, in_=gamma.to_broadcast([128, D]))
        beta_sb = sb.tile([128, D], F32, tag="beta")
        nc.sync.dma_start(out=beta_sb, in_=beta.to_broadcast([128, D]))
        eps_sb = sb.tile([128, 1], F32, tag="eps")
        nc.vector.memset(eps_sb, LN_EPS)

        def emit_body(rep):
            # ---- input loads (ordered for fastest attention start) ----
            xr = xT.rearrange("(c p) s -> c p s", p=128)
            wqr = wq.rearrange("(c p) m -> c p m", p=128)
            wkr = wk.rearrange("(c p) m -> c p m", p=128)
            wvr = wv.rearrange("(c p) m -> c p m", p=128)
            xc = [sb.tile([128, S], BF16, tag=f"xT{c}", name=f"xc{c}")
                  for c in range(KC)]
            # wave 1: first x query-slice as 16 half-tiles (one per DMA engine)
            for c in range(KC):
                nc.sync.dma_start(out=xc[c][:, 0:256], in_=xr[c][:, 0:256])
                nc.sync.dma_start(out=xc[c][:, 256:512], in_=xr[c][:, 256:512])
            # wave 2: all of wk/wq
            wkc = [sb.tile([128, DQ], BF16, tag=f"wk{c}", name=f"wkc{c}")
                   for c in range(KC)]
            wqc = [sb.tile([128, DQ], BF16, tag=f"wq{c}", name=f"wqc{c}")
                   for c in range(KC)]
            for c in range(KC):
                nc.sync.dma_start(out=wkc[c], in_=wkr[c])
                nc.sync.dma_start(out=wqc[c], in_=wqr[c])
            # wave 3: wv
            wvc = []
            for c in range(KC):
                tv = sb.tile([128, DV], BF16, tag=f"wv{c}", name=f"wvc{c}")
                nc.sync.dma_start(out=tv, in_=wvr[c])
                wvc.append(tv)
            for q in range(1, NQT):
                qs = slice(q * 512, (q + 1) * 512)
                for c in range(KC):
                    nc.sync.dma_start(out=xc[c][:, qs], in_=xr[c][:, qs])
            wor = wo.rearrange("(c p) m -> c p m", p=128)
            woc = []
            for c in range(DV // 128):
                t = sb.tile([128, D], BF16, tag=f"wo{c}", name=f"woc{c}")
                nc.sync.dma_start(out=t, in_=wor[c])
                woc.append(t)

            # ---- persistent attention tiles ----
            vones = []
            for c in range(SB):
                t = sb.tile([128, NH_LOC, HD + 1], BF16, tag=f"vo{c}",
                            name=f"vones{c}")
                nc.vector.memset(t[:, :, HD:HD + 1], 1.0)
                vones.append(t)
            kall = sb.tile([128, NH_LOC, S], BF16, tag="kall", name="kall")

            # ---- projection tasks (pool C, [128,512] 1 bank x 2 bufs) ----
            # Each task is a list of ~0.85us steps; steps of one task stay
            # adjacent in the deadline queue so at most 2 C-tiles are open.
            def v_task(c):
                st = {}

                def s1():
                    st["t"] = ps.tile([128, 512], F32, tag="C", bufs=2,
                                      name=f"cv{c}")
                    for d in range(KC):
                        nc.tensor.matmul(st["t"][:, 0:DV],
                                         xc[d][:, c * 128:(c + 1) * 128],
                                         wvc[d],
                                         start=(d == 0), stop=(d == KC - 1))

                def s2():
                    pvr = st["t"][:, 0:DV].rearrange("p (h v) -> p h v",
                                                     h=NH_LOC)
                    nc.vector.tensor_copy(vones[c][:, :, 0:HD], pvr)

                return [s1, s2]

            def kq_task(which, h, csl, outfn):
                # one head x one 512-col slice of x -> [128, 512]
                wc = wkc if which == "k" else wqc
                hc = slice(h * 128, (h + 1) * 128)
                st = {}

                def s1():
                    st["t"] = ps.tile([128, 512], F32, tag="C", bufs=2,
                                      name=f"c{which}{h}")
                    for d in range(4):
                        nc.tensor.matmul(st["t"], wc[d][:, hc], xc[d][:, csl],
                                         start=(d == 0), stop=False)

                def s2():
                    for d in range(4, KC):
                        nc.tensor.matmul(st["t"], wc[d][:, hc], xc[d][:, csl],
                                         start=False, stop=(d == KC - 1))
                    nc.vector.tensor_copy(outfn(), st["t"])

                return [s1, s2]

            def k_task(qs, h):
                ksl = slice(qs * 512, (qs + 1) * 512)
                return kq_task("k", h, ksl, lambda: kall[:, h, ksl])

            def q_task(qt, h, qall):
                qsl = slice(qt * 512, (qt + 1) * 512)
                return kq_task("q", h, qsl, lambda: qall[:, h, :])

            def wo_task(qt, otc, partial, sblk):
                ssl = slice(sblk * 128, (sblk + 1) * 128)
                st = {}

                def half(i):
                    def s():
                        t = ps.tile([128, 512], F32, tag="C", bufs=2,
                                    name=f"cw{qt}_{sblk}_{i}")
                        st[i] = t
                        nsl = slice(i * 512, (i + 1) * 512)
                        nc.tensor.matmul(t, otc[0][:, ssl], woc[0][:, nsl],
                                         start=True, stop=False)
                        nc.tensor.matmul(t, otc[1][:, ssl], woc[1][:, nsl],
                                         start=False, stop=True)
                    return s

                def s3():
                    psb = sb.tile([128, D], BF16, tag="psb", bufs=2, name="psb")
                    nc.vector.tensor_copy(psb[:, 0:512], st[0])
                    nc.vector.tensor_copy(psb[:, 512:1024], st[1])
                    nc.sync.dma_start(out=partial[ssl, :], in_=psb)

                return [half(0), half(1), s3]

            # ---- deadline queue of steps ----
            import bisect

            queue = []  # sorted (deadline_slot, seq, step_fn, earliest_slot)
            seq_ctr = [0]

            def enqueue(deadline, steps, earliest=0):
                for fn in steps:
                    bisect.insort(queue, (deadline, seq_ctr[0], fn, earliest))
                    seq_ctr[0] += 1

            def pop_steps(slot, budget=1):
                n = 0
                while queue and n < 6:
                    deadline, _, fn, earliest = queue[0]
                    if deadline <= slot + 1 or (n < budget and earliest <= slot):
                        queue.pop(0)
                        fn()
                        n += 1
                    else:
                        break

            def drain():
                while queue:
                    _, _, fn, _ = queue.pop(0)
                    fn()

            # ---- attention for one (head, qt): 16 key chunks ----
            def attention(h, qt, qall, otc):
                slot0 = qt * 64 + h * 16
                po = ps.tile([128, 1024], F32, tag="B", bufs=1,
                             name=f"po{h}_{qt}")
                for c in range(SB):
                    ksl = slice(c * 128, (c + 1) * 128)
                    pa = ps.tile([128, 1024], F32, tag="A", bufs=2,
                                 name=f"pa{h}_{qt}_{c}")
                    nc.tensor.matmul(pa[:, 0:512], kall[0:HD, h, ksl],
                                     qall[0:HD, h, :])
                    nc.tensor.matmul(pa[:, 512:1024], kall[HD:128, h, ksl],
                                     qall[HD:128, h, :])
                    e12 = sb.tile([128, 1024], BF16, tag="e12", bufs=4,
                                  name="e12")
                    nc.scalar.activation(out=e12, in_=pa,
                                         func=mybir.ActivationFunctionType.Exp,
                                         scale=SCALE)
                    nc.tensor.matmul(po[0:HD + 1, 0:512], vones[c][:, h, :],
                                     e12[:, 0:512],
                                     start=(c == 0), stop=(c == SB - 1))
                    nc.tensor.matmul(po[0:HD + 1, 512:1024], vones[c][:, h, :],
                                     e12[:, 512:1024],
                                     start=(c == 0), stop=(c == SB - 1))
                    pop_steps(slot0 + c)
                # fast evac of raw PV sums (frees PSUM B), then combine from
                # SBUF: divide by broadcast sums, branch2 pre-scaled 1/lambda
                pe_sb = sb.tile([HD + 1, 1024], F32, tag="pesb", bufs=2,
                                name="pesb")
                nc.vector.tensor_copy(pe_sb, po[0:HD + 1, :])
                srow = sb.tile([1, 1024], F32, tag="srow", bufs=2,
                               name="srow")
                nc.vector.tensor_copy(srow, pe_sb[HD:HD + 1, :])
                nc.vector.reciprocal(srow, srow)
                nc.vector.tensor_scalar_mul(srow[0:1, 512:1024],
                                            srow[0:1, 512:1024],
                                            lam_sb[0:1, :])
                rbc = sb.tile([HD, 1024], F32, tag="rbc", bufs=2, name="rbc")
                nc.gpsimd.partition_broadcast(rbc, srow, channels=HD)
                t2 = sb.tile([HD, 512], F32, tag="t2", bufs=2, name="t2")
                t1 = sb.tile([HD, 512], F32, tag="t1", bufs=2, name="t1")
                nc.vector.tensor_mul(t2, pe_sb[0:HD, 512:1024],
                                     rbc[:, 512:1024])
                nc.vector.tensor_mul(t1, pe_sb[0:HD, 0:512], rbc[:, 0:512])
                nc.vector.tensor_sub(t1, t1, t2)
                osl = otc[h // 2][(h % 2) * HD:(h % 2) * HD + HD, :]
                nc.scalar.copy(osl, t1)

            def rs_issue(qt, partial):
                red = dram.tile([128, D], BF16, tag=f"red{qt}",
                                name=f"red{qt}_{rep}")
                if single or nocc:
                    nc.sync.dma_start(out=red[:, :], in_=partial[0:128, :])
                else:
                    nc.gpsimd.collective_compute(
                        "ReduceScatter",
                        mybir.AluOpType.add,
                        replica_groups=[[0, 1, 2, 3], [4, 5, 6, 7]],
                        ins=[partial.opt()],
                        outs=[red.opt()],
                    )
                return red

            def ln_work(qt, red):
                xb = sb.tile([128, D], BF16, tag="lnxb", bufs=2, name="xb")
                nc.sync.dma_start(out=xb, in_=red[:, :])
                xt = sb.tile([128, D], F32, tag="lnx", bufs=2, name="xt")
                nc.vector.tensor_copy(xt, xb)
                xrr = xt.rearrange("p (a b) -> p a b", b=512)
                st = sb.tile([128, 2, 6], F32, tag="st", bufs=2, name="st")
                nc.vector.bn_stats(out=st[:, 0, :], in_=xrr[:, 0, :])
                nc.vector.bn_stats(out=st[:, 1, :], in_=xrr[:, 1, :])
                mv = sb.tile([128, 2], F32, tag="mv", bufs=2, name="mv")
                nc.vector.bn_aggr(out=mv, in_=st)
                rstd = sb.tile([128, 1], F32, tag="rstd", bufs=2, name="rstd")
                nc.scalar.activation(out=rstd, in_=mv[:, 1:2],
                                     func=mybir.ActivationFunctionType.Sqrt,
                                     bias=eps_sb, scale=1.0)
                nc.vector.reciprocal(rstd, rstd)
                ot = sb.tile([128, D], F32, tag="lno", bufs=2, name="ot")
                nc.vector.tensor_scalar(ot, xt, mv[:, 0:1], rstd,
                                        op0=mybir.AluOpType.subtract,
                                        op1=mybir.AluOpType.mult)
                nc.vector.tensor_mul(ot, ot, gamma_sb)
                nc.vector.tensor_add(ot, ot, beta_sb)
                nc.sync.dma_start(out=out[qt * 128:(qt + 1) * 128, :], in_=ot)

            # ---- emission schedule ----
            # prefix: just enough to start h0/qt0; everything else rides the
            # deadline queue, popped one ~0.85us step per attention chunk so
            # the exp stream (ACT) never starves.
            qall0 = sb.tile([128, NH_LOC, 512], BF16, tag="qall", bufs=2,
                            name="qall0")
            for st in k_task(0, 0) + q_task(0, 0, qall0) + v_task(0):
                st()

            # remaining qt0 work, deadline = slot of first consumer - 1
            for c in range(1, SB):
                enqueue(max(c - 2, 0), v_task(c))
            for h in range(NH_LOC):
                for qs in range(NQT):
                    if h == 0 and qs == 0:
                        continue
                    enqueue(max(h * 16 + 4 * qs - 2, 0), k_task(qs, h))
            for h in range(1, NH_LOC):
                enqueue(h * 16 - 2, q_task(0, h, qall0))

            qall = qall0
            for qt in range(NQT):
                otc = [sb.tile([128, 512], BF16, tag=f"otc{i}", bufs=2,
                               name=f"otc{i}_{qt}") for i in range(2)]
                if qt + 1 < NQT:
                    qall_n = sb.tile([128, NH_LOC, 512], BF16, tag="qall",
                                     bufs=2, name=f"qall{qt + 1}")
                    for h in range(NH_LOC):
                        enqueue((qt + 1) * 64 + h * 16 - 12,
                                q_task(qt + 1, h, qall_n))
                else:
                    qall_n = None
                for h in range(NH_LOC):
                    attention(h, qt, qall, otc)
                # Wo partial + RS + LN ride the next qt's queue (deadlines
                # right after this qt ends); last qt drains directly.
                partial = dram.tile([512, D], BF16, tag=f"partial{qt}",
                                    name=f"partial{qt}_{rep}")
                base = (qt + 1) * 64
                for sblk in range(4):
                    enqueue(base + 28 + 2 * sblk, wo_task(qt, otc, partial, sblk),
                            earliest=base + 20 + 2 * sblk)
                reds = {}
                enqueue(base + 38,
                        [lambda q=qt, p=partial: reds.__setitem__(q, rs_issue(q, p))],
                        earliest=base + 36)
                enqueue(base + 62, [lambda q=qt: ln_work(q, reds[q])],
                        earliest=base + 60)
                qall = qall_n

            drain()

        for _rep in range(repeat):
            emit_body(_rep)

    nc.compile()
    return nc


def _shard(inputs):
    bf16 = ml_dtypes.bfloat16
    x = np.asarray(inputs["x"], dtype=np.float32)
    Wq = np.asarray(inputs["Wq"], dtype=np.float32)
    Wk = np.asarray(inputs["Wk"], dtype=np.float32)
    Wv = np.asarray(inputs["Wv"], dtype=np.float32)
    Wo = np.asarray(inputs["Wo"], dtype=np.float32)
    gamma = np.asarray(inputs["gamma"], dtype=np.float32).reshape(1, D)
    beta = np.asarray(inputs["beta"], dtype=np.float32).reshape(1, D)
    lq1 = np.asarray(inputs["lambda_q1"], dtype=np.float32)
    lk1 = np.asarray(inputs["lambda_k1"], dtype=np.float32)
    lq2 = np.asarray(inputs["lambda_q2"], dtype=np.float32)
    lk2 = np.asarray(inputs["lambda_k2"], dtype=np.float32)
    lam = (np.exp(np.sum(lq1 * lk1, dtype=np.float32), dtype=np.float32)
           - np.exp(np.sum(lq2 * lk2, dtype=np.float32), dtype=np.float32)
           + np.float32(LAMBDA_INIT)).reshape(1, 1).astype(np.float32)

    wq_h = Wq.reshape(D, NH, 2 * HD)
    wk_h = Wk.reshape(D, NH, 2 * HD)
    wv_h = Wv.reshape(D, NH, HD)
    wo_h = Wo.reshape(NH, HD, D)

    xTs = [np.ascontiguousarray(x[b].T).astype(bf16) for b in range(B)]
    in_maps = []
    for c in range(N_CORES):
        b = c // GRP
        hg = c % GRP
        hs = slice(hg * NH_LOC, (hg + 1) * NH_LOC)
        in_maps.append({
            "xT": xTs[b],
            "wq": np.ascontiguousarray(wq_h[:, hs, :].reshape(D, DQ)).astype(bf16),
            "wk": np.ascontiguousarray(wk_h[:, hs, :].reshape(D, DQ)).astype(bf16),
            "wv": np.ascontiguousarray(wv_h[:, hs, :].reshape(D, DV)).astype(bf16),
            "wo": np.ascontiguousarray(wo_h[hs].reshape(DV, D)).astype(bf16),
            "lam": lam,
            "gamma": gamma,
            "beta": beta,
        })
    return in_maps


def _unshard(results):
    out = np.empty((B, S, D), dtype=np.float32)
    for c in range(N_CORES):
        b = c // GRP
        r = c % GRP
        res = results[c]["out"] if isinstance(results[c], dict) else results[c]
        for qt in range(NQT):
            out[b, qt * 512 + r * 128:qt * 512 + (r + 1) * 128, :] = \
                res[qt * 128:(qt + 1) * 128, :]
    return out


def run_all(trace=False, repeat=1, fast=FAST_DEFAULT, nocc=False, **inputs):
    key = (repeat, fast, nocc)
    if key not in _CACHE:
        _CACHE[key] = _build(repeat=repeat, fast=fast, nocc=nocc)
    nc = _CACHE[key]
    in_maps = _shard(inputs)
    res = bass_utils.run_bass_kernel_spmd(
        nc, in_maps, core_ids=list(range(N_CORES)), trace=trace)
    return _unshard(res.results), res


def kernel(**inputs):
    out, _ = run_all(trace=False, **inputs)
    return out


# revision 28
# speedup vs baseline: 1.0035x; 1.0035x over previous
"""MultiHeadDiffAttention Trainium2 kernel (8 NeuronCores), v3.

Sharding: data-parallel over batch (B=2 -> 2 groups of 4 cores), tensor-parallel
over heads within a group (16 heads -> 4 heads/core).

Pipeline (per core, SPMD uniform):
  - all storage bf16 (x, weights, K/Q/V tiles, exp weights, attention out);
    matmul accumulation f32 in PSUM.
  - qt-outer / head-inner attention: per 512-query tile, per head, stream 16
    key chunks: scores matmul -> exp (the ONLY steady ACT work, [128,1024]
    per chunk) -> P^T@V matmul with a ones column riding along for the
    softmax sums.
  - combine: evacuate raw PV sums fast (releases the PSUM accumulator),
    reciprocal the sums row, lambda-scale branch 2, gpsimd broadcast, two
    multiplies + subtract on DVE, ACT copy places the head rows into otc.
  - per-qt Wo partial [512, D] bf16 -> ReduceScatter over the 4-core group
    (issued early in the next qt, overlapped) -> per-qt LayerNorm -> out.
  - PSUM: pool A [128,1024]x2 (scores), pool B [128,1024]x1 (PV accum),
    pool C [128,512]x2 (projections/Wo) = 8 banks.
  - projections/Wo/RS/LN ride a deadline-sorted step queue, popped one
    ~0.85us step per attention chunk so the exp stream never starves and
    the in-order engine queues avoid priority inversion.

Output rows per core: row (qt*128 + i) = batch row (qt*512 + rank*128 + i).
"""

import math
from contextlib import ExitStack

import numpy as np
import ml_dtypes

import concourse.bass as bass
import concourse.mybir as mybir
import concourse.tile as tile
from concourse import bacc
from concourse import bass_utils

F32 = mybir.dt.float32
BF16 = mybir.dt.bfloat16

B = 2
S = 2048
D = 1024
NH = 16
HD = 64
N_CORES = 8
GRP = N_CORES // B  # 4 cores per batch group
NH_LOC = NH // GRP  # 4 heads per core
DQ = NH_LOC * 2 * HD  # 512 local q/k projection width
DV = NH_LOC * HD  # 256 local v projection width
LAYER_IDX = 12
LAMBDA_INIT = 0.8 - 0.6 * math.exp(-0.3 * (LAYER_IDX - 1))
LN_EPS = 1e-5
SCALE = HD ** (-0.5)

KC = D // 128  # 8 contraction chunks for projections
SB = S // 128  # 16 key chunks
NQT = S // 512  # 4 query tiles

_CACHE = {}
FAST_DEFAULT = False


def _build(repeat=1, single=False, fast=False, nocc=False):
    nc = bacc.Bacc("TRN2", target_bir_lowering=False, debug=False,
                   num_devices=1 if single else N_CORES)

    xT = nc.dram_tensor("xT", [D, S], BF16, kind="ExternalInput").ap()
    wq = nc.dram_tensor("wq", [D, DQ], BF16, kind="ExternalInput").ap()
    wk = nc.dram_tensor("wk", [D, DQ], BF16, kind="ExternalInput").ap()
    wv = nc.dram_tensor("wv", [D, DV], BF16, kind="ExternalInput").ap()
    wo = nc.dram_tensor("wo", [DV, D], BF16, kind="ExternalInput").ap()
    lam = nc.dram_tensor("lam", [1, 1], F32, kind="ExternalInput").ap()
    gamma = nc.dram_tensor("gamma", [1, D], F32, kind="ExternalInput").ap()
    beta = nc.dram_tensor("beta", [1, D], F32, kind="ExternalInput").ap()
    out = nc.dram_tensor("out", [S // 4, D], F32, kind="ExternalOutput").ap()

    with tile.TileContext(nc) as tc, ExitStack() as ctx:
        sb = ctx.enter_context(tc.tile_pool(name="sb", bufs=1))
        ps = ctx.enter_context(tc.tile_pool(name="ps", bufs=1, space="PSUM"))
        dram = ctx.enter_context(tc.tile_pool(name="dram", bufs=1, space="DRAM"))

        # ---- constants ----
        lam_sb = sb.tile([128, 1], F32, tag="lam")
        nc.sync.dma_start(out=lam_sb, in_=lam.to_broadcast([128, 1]))
        gamma_sb = sb.tile([128, D], F32, tag="gamma")
        nc.sync.dma_start(out=gamma_sb, in_=gamma.to_broadcast([128, D]))
        beta_sb = sb.tile([128, D], F32, tag="beta")
        nc.sync.dma_start(out=beta_sb, in_=beta.to_broadcast([128, D]))
        eps_sb = sb.tile([128, 1], F32, tag="eps")
        nc.vector.memset(eps_sb, LN_EPS)

        def emit_body(rep):
            # ---- input loads (ordered for fastest attention start) ----
            xr = xT.rearrange("(c p) s -> c p s", p=128)
            wqr = wq.rearrange("(c p) m -> c p m", p=128)
            wkr = wk.rearrange("(c p) m -> c p m", p=128)
            wvr = wv.rearrange("(c p) m -> c p m", p=128)
            xc = [sb.tile([128, S], BF16, tag=f"xT{c}", name=f"xc{c}")
                  for c in range(KC)]
            qs0 = slice(0, 512)
            for c in range(KC):
                nc.sync.dma_start(out=xc[c][:, qs0], in_=xr[c][:, qs0])
            wkc = [sb.tile([128, DQ], BF16, tag=f"wk{c}", name=f"wkc{c}")
                   for c in range(KC)]
            wqc = [sb.tile([128, DQ], BF16, tag=f"wq{c}", name=f"wqc{c}")
                   for c in range(KC)]
            wvc = []
            for c in range(KC):
                tv = sb.tile([128, DV], BF16, tag=f"wv{c}", name=f"wvc{c}")
                nc.sync.dma_start(out=tv, in_=wvr[c])
                wvc.append(tv)
            for c in range(4):
                nc.sync.dma_start(out=wkc[c], in_=wkr[c])
            for c in range(4):
                nc.sync.dma_start(out=wqc[c], in_=wqr[c])
            for c in range(4, KC):
                nc.sync.dma_start(out=wkc[c], in_=wkr[c])
            for c in range(4, KC):
                nc.sync.dma_start(out=wqc[c], in_=wqr[c])
            for q in range(1, NQT):
                qs = slice(q * 512, (q + 1) * 512)
                for c in range(KC):
                    nc.sync.dma_start(out=xc[c][:, qs], in_=xr[c][:, qs])
            wor = wo.rearrange("(c p) m -> c p m", p=128)
            woc = []
            for c in range(DV // 128):
                t = sb.tile([128, D], BF16, tag=f"wo{c}", name=f"woc{c}")
                nc.sync.dma_start(out=t, in_=wor[c])
                woc.append(t)

            # ---- persistent attention tiles ----
            vones = []
            for c in range(SB):
                t = sb.tile([128, NH_LOC, HD + 1], BF16, tag=f"vo{c}",
                            name=f"vones{c}")
                nc.vector.memset(t[:, :, HD:HD + 1], 1.0)
                vones.append(t)
            kall = sb.tile([128, NH_LOC, S], BF16, tag="kall", name="kall")

            # ---- projection tasks (pool C, [128,512] 1 bank x 2 bufs) ----
            # Each task is a list of ~0.85us steps; steps of one task stay
            # adjacent in the deadline queue so at most 2 C-tiles are open.
            def v_task(c):
                st = {}

                def s1():
                    st["t"] = ps.tile([128, 512], F32, tag="C", bufs=2,
                                      name=f"cv{c}")
                    for d in range(KC):
                        nc.tensor.matmul(st["t"][:, 0:DV],
                                         xc[d][:, c * 128:(c + 1) * 128],
                                         wvc[d],
                                         start=(d == 0), stop=(d == KC - 1))

                def s2():
                    pvr = st["t"][:, 0:DV].rearrange("p (h v) -> p h v",
                                                     h=NH_LOC)
                    nc.vector.tensor_copy(vones[c][:, :, 0:HD], pvr)

                return [s1, s2]

            def kq_task(which, h, csl, outfn):
                # one head x one 512-col slice of x -> [128, 512]
                wc = wkc if which == "k" else wqc
                hc = slice(h * 128, (h + 1) * 128)
                st = {}

                def s1():
                    st["t"] = ps.tile([128, 512], F32, tag="C", bufs=2,
                                      name=f"c{which}{h}")
                    for d in range(4):
                        nc.tensor.matmul(st["t"], wc[d][:, hc], xc[d][:, csl],
                                         start=(d == 0), stop=False)

                def s2():
                    for d in range(4, KC):
                        nc.tensor.matmul(st["t"], wc[d][:, hc], xc[d][:, csl],
                                         start=False, stop=(d == KC - 1))
                    nc.vector.tensor_copy(outfn(), st["t"])

                return [s1, s2]

            def k_task(qs, h):
                ksl = slice(qs * 512, (qs + 1) * 512)
                return kq_task("k", h, ksl, lambda: kall[:, h, ksl])

            def q_task(qt, h, qall):
                qsl = slice(qt * 512, (qt + 1) * 512)
                return kq_task("q", h, qsl, lambda: qall[:, h, :])

            def wo_task(qt, otc, partial, sblk):
                ssl = slice(sblk * 128, (sblk + 1) * 128)
                st = {}

                def half(i):
                    def s():
                        t = ps.tile([128, 512], F32, tag="C", bufs=2,
                                    name=f"cw{qt}_{sblk}_{i}")
                        st[i] = t
                        nsl = slice(i * 512, (i + 1) * 512)
                        nc.tensor.matmul(t, otc[0][:, ssl], woc[0][:, nsl],
                                         start=True, stop=False)
                        nc.tensor.matmul(t, otc[1][:, ssl], woc[1][:, nsl],
                                         start=False, stop=True)
                    return s

                def s3():
                    psb = sb.tile([128, D], BF16, tag="psb", bufs=2, name="psb")
                    nc.vector.tensor_copy(psb[:, 0:512], st[0])
                    nc.vector.tensor_copy(psb[:, 512:1024], st[1])
                    nc.sync.dma_start(out=partial[ssl, :], in_=psb)

                return [half(0), half(1), s3]

            # ---- deadline queue of steps ----
            import bisect

            queue = []  # sorted (deadline_slot, seq, step_fn, earliest_slot)
            seq_ctr = [0]

            def enqueue(deadline, steps, earliest=0):
                for fn in steps:
                    bisect.insort(queue, (deadline, seq_ctr[0], fn, earliest))
                    seq_ctr[0] += 1

            def pop_steps(slot, budget=1):
                n = 0
                while queue and n < 6:
                    deadline, _, fn, earliest = queue[0]
                    if deadline <= slot + 1 or (n < budget and earliest <= slot):
                        queue.pop(0)
                        fn()
                        n += 1
                    else:
                        break

            def drain():
                while queue:
                    _, _, fn, _ = queue.pop(0)
                    fn()

            # ---- attention for one (head, qt): 16 key chunks ----
            def attention(h, qt, qall, otc):
                slot0 = qt * 64 + h * 16
                po = ps.tile([128, 1024], F32, tag="B", bufs=1,
                             name=f"po{h}_{qt}")
                for c in range(SB):
                    ksl = slice(c * 128, (c + 1) * 128)
                    pa = ps.tile([128, 1024], F32, tag="A", bufs=2,
                                 name=f"pa{h}_{qt}_{c}")
                    nc.tensor.matmul(pa[:, 0:512], kall[0:HD, h, ksl],
                                     qall[0:HD, h, :])
                    nc.tensor.matmul(pa[:, 512:1024], kall[HD:128, h, ksl],
                                     qall[HD:128, h, :])
                    e12 = sb.tile([128, 1024], BF16, tag="e12", bufs=4,
                                  name="e12")
                    nc.scalar.activation(out=e12, in_=pa,
                                         func=mybir.ActivationFunctionType.Exp,
                                         scale=SCALE)
                    nc.tensor.matmul(po[0:HD + 1, 0:512], vones[c][:, h, :],
                                     e12[:, 0:512],
                                     start=(c == 0), stop=(c == SB - 1))
                    nc.tensor.matmul(po[0:HD + 1, 512:1024], vones[c][:, h, :],
                                     e12[:, 512:1024],
                                     start=(c == 0), stop=(c == SB - 1))
                    pop_steps(slot0 + c)
                # fast evac of raw PV sums (frees PSUM B), then combine from
                # SBUF: divide by broadcast sums, branch2 pre-scaled 1/lambda
                pe_sb = sb.tile([HD + 1, 1024], F32, tag="pesb", bufs=2,
                                name="pesb")
                nc.vector.tensor_copy(pe_sb, po[0:HD + 1, :])
                srow = sb.tile([1, 1024], F32, tag="srow", bufs=2,
                               name="srow")
                nc.vector.tensor_copy(srow, pe_sb[HD:HD + 1, :])
                nc.vector.reciprocal(srow, srow)
                nc.vector.tensor_scalar_mul(srow[0:1, 512:1024],
                                            srow[0:1, 512:1024],
                                            lam_sb[0:1, :])
                rbc = sb.tile([HD, 1024], F32, tag="rbc", bufs=2, name="rbc")
                nc.gpsimd.partition_broadcast(rbc, srow, channels=HD)
                t2 = sb.tile([HD, 512], F32, tag="t2", bufs=2, name="t2")
                t1 = sb.tile([HD, 512], F32, tag="t1", bufs=2, name="t1")
                nc.vector.tensor_mul(t2, pe_sb[0:HD, 512:1024],
                                     rbc[:, 512:1024])
                nc.vector.tensor_mul(t1, pe_sb[0:HD, 0:512], rbc[:, 0:512])
                nc.vector.tensor_sub(t1, t1, t2)
                osl = otc[h // 2][(h % 2) * HD:(h % 2) * HD + HD, :]
                nc.scalar.copy(osl, t1)

            def rs_issue(qt, partial):
                red = dram.tile([128, D], BF16, tag=f"red{qt}",
                                name=f"red{qt}_{rep}")
                if single or nocc:
                    nc.sync.dma_start(out=red[:, :], in_=partial[0:128, :])
                else:
                    nc.gpsimd.collective_compute(
                        "ReduceScatter",
                        mybir.AluOpType.add,
                        replica_groups=[[0, 1, 2, 3], [4, 5, 6, 7]],
                        ins=[partial.opt()],
                        outs=[red.opt()],
                    )
                return red

            def ln_work(qt, red):
                xb = sb.tile([128, D], BF16, tag="lnxb", bufs=2, name="xb")
                nc.sync.dma_start(out=xb, in_=red[:, :])
                xt = sb.tile([128, D], F32, tag="lnx", bufs=2, name="xt")
                nc.vector.tensor_copy(xt, xb)
                xrr = xt.rearrange("p (a b) -> p a b", b=512)
                st = sb.tile([128, 2, 6], F32, tag="st", bufs=2, name="st")
                nc.vector.bn_stats(out=st[:, 0, :], in_=xrr[:, 0, :])
                nc.vector.bn_stats(out=st[:, 1, :], in_=xrr[:, 1, :])
                mv = sb.tile([128, 2], F32, tag="mv", bufs=2, name="mv")
                nc.vector.bn_aggr(out=mv, in_=st)
                rstd = sb.tile([128, 1], F32, tag="rstd", bufs=2, name="rstd")
                nc.scalar.activation(out=rstd, in_=mv[:, 1:2],
                                     func=mybir.ActivationFunctionType.Sqrt,
                                     bias=eps_sb, scale=1.0)
                nc.vector.reciprocal(rstd, rstd)
                ot = sb.tile([128, D], F32, tag="lno", bufs=2, name="ot")
                nc.vector.tensor_scalar(ot, xt, mv[:, 0:1], rstd,
                                        op0=mybir.AluOpType.subtract,
                                        op1=mybir.AluOpType.mult)
                nc.vector.tensor_mul(ot, ot, gamma_sb)
                nc.vector.tensor_add(ot, ot, beta_sb)
                nc.sync.dma_start(out=out[qt * 128:(qt + 1) * 128, :], in_=ot)

            # ---- emission schedule ----
            # prefix: just enough to start h0/qt0; everything else rides the
            # deadline queue, popped one ~0.85us step per attention chunk so
            # the exp stream (ACT) never starves.
            qall0 = sb.tile([128, NH_LOC, 512], BF16, tag="qall", bufs=2,
                            name="qall0")
            for st in v_task(0) + k_task(0, 0) + q_task(0, 0, qall0):
                st()

            # remaining qt0 work, deadline = slot of first consumer - 1
            for c in range(1, SB):
                enqueue(max(c - 2, 0), v_task(c))
            for h in range(NH_LOC):
                for qs in range(NQT):
                    if h == 0 and qs == 0:
                        continue
                    enqueue(max(h * 16 + 4 * qs - 2, 0), k_task(qs, h))
            for h in range(1, NH_LOC):
                enqueue(h * 16 - 2, q_task(0, h, qall0))

            qall = qall0
            for qt in range(NQT):
                otc = [sb.tile([128, 512], BF16, tag=f"otc{i}", bufs=2,
                               name=f"otc{i}_{qt}") for i in range(2)]
                if qt + 1 < NQT:
                    qall_n = sb.tile([128, NH_LOC, 512], BF16, tag="qall",
                                     bufs=2, name=f"qall{qt + 1}")
                    for h in range(NH_LOC):
                        enqueue((qt + 1) * 64 + h * 16 - 12,
                                q_task(qt + 1, h, qall_n))
                else:
                    qall_n = None
                for h in range(NH_LOC):
                    attention(h, qt, qall, otc)
                # Wo partial + RS + LN ride the next qt's queue (deadlines
                # right after this qt ends); last qt drains directly.
                partial = dram.tile([512, D], BF16, tag=f"partial{qt}",
                                    name=f"partial{qt}_{rep}")
                base = (qt + 1) * 64
                for sblk in range(4):
                    enqueue(base + 28 + 2 * sblk, wo_task(qt, otc, partial, sblk),
                            earliest=base + 20 + 2 * sblk)
                reds = {}
                enqueue(base + 38,
                        [lambda q=qt, p=partial: reds.__setitem__(q, rs_issue(q, p))],
                        earliest=base + 36)
                enqueue(base + 62, [lambda q=qt: ln_work(q, reds[q])],
                        earliest=base + 60)
                qall = qall_n

            drain()

        for _rep in range(repeat):
            emit_body(_rep)

    nc.compile()
    return nc


def _shard(inputs):
    bf16 = ml_dtypes.bfloat16
    x = np.asarray(inputs["x"], dtype=np.float32)
    Wq = np.asarray(inputs["Wq"], dtype=np.float32)
    Wk = np.asarray(inputs["Wk"], dtype=np.float32)
    Wv = np.asarray(inputs["Wv"], dtype=np.float32)
    Wo = np.asarray(inputs["Wo"], dtype=np.float32)
    gamma = np.asarray(inputs["gamma"], dtype=np.float32).reshape(1, D)
    beta = np.asarray(inputs["beta"], dtype=np.float32).reshape(1, D)
    lq1 = np.asarray(inputs["lambda_q1"], dtype=np.float32)
    lk1 = np.asarray(inputs["lambda_k1"], dtype=np.float32)
    lq2 = np.asarray(inputs["lambda_q2"], dtype=np.float32)
    lk2 = np.asarray(inputs["lambda_k2"], dtype=np.float32)
    lam = (np.exp(np.sum(lq1 * lk1, dtype=np.float32), dtype=np.float32)
           - np.exp(np.sum(lq2 * lk2, dtype=np.float32), dtype=np.float32)
           + np.float32(LAMBDA_INIT)).reshape(1, 1).astype(np.float32)

    wq_h = Wq.reshape(D, NH, 2 * HD)
    wk_h = Wk.reshape(D, NH, 2 * HD)
    wv_h = Wv.reshape(D, NH, HD)
    wo_h = Wo.reshape(NH, HD, D)

    xTs = [np.ascontiguousarray(x[b].T).astype(bf16) for b in range(B)]
    in_maps = []
    for c in range(N_CORES):
        b = c // GRP
        hg = c % GRP
        hs = slice(hg * NH_LOC, (hg + 1) * NH_LOC)
        in_maps.append({
            "xT": xTs[b],
            "wq": np.ascontiguousarray(wq_h[:, hs, :].reshape(D, DQ)).astype(bf16),
            "wk": np.ascontiguousarray(wk_h[:, hs, :].reshape(D, DQ)).astype(bf16),
            "wv": np.ascontiguousarray(wv_h[:, hs, :].reshape(D, DV)).astype(bf16),
            "wo": np.ascontiguousarray(wo_h[hs].reshape(DV, D)).astype(bf16),
            "lam": lam,
            "gamma": gamma,
            "beta": beta,
        })
    return in_maps


def _unshard(results):
    out = np.empty((B, S, D), dtype=np.float32)
    for c in range(N_CORES):
        b = c // GRP
        r = c % GRP
        res = results[c]["out"] if isinstance(results[c], dict) else results[c]
        for qt in range(NQT):
            out[b, qt * 512 + r * 128:qt * 512 + (r + 1) * 128, :] = \
                res[qt * 128:(qt + 1) * 128, :]
    return out


def run_all(trace=False, repeat=1, fast=FAST_DEFAULT, nocc=False, **inputs):
    key = (repeat, fast, nocc)
    if key not in _CACHE:
        _CACHE[key] = _build(repeat=repeat, fast=fast, nocc=nocc)
    nc = _CACHE[key]
    in_maps = _shard(inputs)
    res = bass_utils.run_bass_kernel_spmd(
        nc, in_maps, core_ids=list(range(N_CORES)), trace=trace)
    return _unshard(res.results), res


def kernel(**inputs):
    out, _ = run_all(trace=False, **inputs)
    return out
